# revision 19
# baseline (speedup 1.0000x reference)
"""DeformConv1d Trainium2 Bass kernel (optimized).

Problem: x[4,512,4096] f32, offsets[4,1,4090,7] f32, weight[512,512,7], bias[512]
  T[b,o,k]   = clamp(o + k + offsets[b,0,o,k], o, o+6)
  samp[b,c,o,k] = linear-interp of x[b,c,:] at T
  out[b,d,o] = sum_{c,k} samp[b,c,o,k] * weight[d,c,k] + bias[d]

Key identity: the clamp keeps every sample inside [o, o+6], so output o only
touches taps p in [o, o+7], and the tap weight of p is relu(1 - |p - T|).
With an o-tile of 121 the tap band is exactly 128 wide:

  out[o,d] = sum_{k, dp<128} S_k[dp, o] * Y[o0+dp, k, d] (+ bias via Y_0)
    S_k[dp,o] = relu(1 - |(dp - (o-o0)) - c_k[o]|),  c_k[o] = clamp(k + off, 0, 6)
    Y[p,k,d]  = sum_c x[c, p] * weight[d, c, k]

Both stages are dense bf16 matmuls (f32 PSUM accumulate); the 595-matmul
stream runs at the 216ns/matmul N=512 streaming roofline.  Design notes:
  - c_k is precomputed on host into per-tile rows (c2); a 0-stride DMA
    expands each row across partitions; S = relu(1-|c-diag|) on DVE+ACT.
  - Stage 1 runs k-outer over groups of 6 o-tiles so the first tile does
    not need the whole 3.7MB weight within one tile period; S-builds are
    interleaved between k-sweeps so they sit early in the strict-FIFO
    DVE/ACT queues and are ready when stage 2 arrives.
  - All bulk traffic rides the gpsimd SW DGE queue, which sustains
    ~400GB/s with per-partition-contiguous "slab" layouts (the HW DGE
    queues measure only ~50GB/s here).  Weight is host-packed as per-k
    slabs [k, p, ci*d] (4KB contiguous rows on both DMA sides).
  - A 164KB warm tile on the scalar HW queue feeds 4 dummy matmuls that
    hold the PE busy through the HAM activity window, so the real stream
    runs at 2.4GHz (warm) from the start.
  - bias is folded into the k=0 Y eviction (sum_p S_k[p,o] == 1); output
    is stored bf16 (host upcasts); the last tile's stage 2 runs in
    d-halves so its eviction/DMA overlaps the final matmuls.
Sharding: 8 cores = 4 batches x 2 halves of out_len (data parallel, no comm).
"""

import os
import sys

import ml_dtypes
import numpy as np

for _p in ("/opt/trn_rl_repo", os.path.expanduser("~/.axon_site/_ro/trn_rl_repo")):
    if os.path.isdir(_p) and _p not in sys.path:
        sys.path.insert(0, _p)

import concourse.mybir as mybir
import concourse.tile as tile
from concourse import bacc
from concourse.bass_utils import run_bass_kernel_spmd

B, CIN, COUT, L, K = 4, 512, 512, 4096, 7
OUT_LEN = 4090
HALF = 2045          # out positions per core (2 halves per batch)
OT = 121             # o-tile size -> tap band = OT + 7 = 128
TILES = 17           # 17 * 121 = 2057 >= 2045
OPAD = TILES * OT    # 2057 padded out positions per core
XW = (TILES - 1) * OT + 128  # 2064: rightmost x column any tile reads
P = 128
NCK = CIN // P       # 4 c-chunks
CW = K * OT + 1      # 848: packed c/diag/S row width (847 used, +1 pad)
G = 6                # o-tiles per stage-1 k-outer group
F32 = mybir.dt.float32
BF16 = mybir.dt.bfloat16

_prog_cache = {}


def _build_program():
    nc = bacc.Bacc("TRN2", target_bir_lowering=False, debug=False)

    xs_d = nc.dram_tensor("xs", [CIN, XW], BF16, kind="ExternalInput")
    wt_d = nc.dram_tensor("wt", [K, P, NCK * COUT], BF16, kind="ExternalInput")
    c2_d = nc.dram_tensor("c2", [TILES, CW], F32, kind="ExternalInput")
    diag_d = nc.dram_tensor("diag7", [P, CW], F32, kind="ExternalInput")
    bias_d = nc.dram_tensor("biasr", [1, COUT], F32, kind="ExternalInput")
    warm_d = nc.dram_tensor("warm", [P, P + COUT], BF16, kind="ExternalInput")
    out_d = nc.dram_tensor("out", [OPAD, COUT], BF16, kind="ExternalOutput")

    groups = [list(range(g, min(g + G, TILES))) for g in range(0, TILES, G)]

    with tile.TileContext(nc) as tc:
        with (
            tc.tile_pool(name="const", bufs=1) as cpool,
            tc.tile_pool(name="cbt", bufs=TILES + 1) as cbpool,
            tc.tile_pool(name="stiles", bufs=G + 1) as stpool,
            tc.tile_pool(name="ytiles", bufs=G + 1) as ypool,
            tc.tile_pool(name="otiles", bufs=3) as opool,
            tc.tile_pool(name="pwarm", bufs=1, space="PSUM") as pwarm,
            tc.tile_pool(name="psy", bufs=4, space="PSUM") as psy,
            tc.tile_pool(name="pso", bufs=3, space="PSUM") as pso,
        ):
            # ---- bulk inputs + broadcasts all on the gpsimd SW queue
            # (measured ~250GB/s vs ~50GB/s on the HW DGE queues), issued in
            # group-0 consumption-deadline order; consts on scalar HW queue;
            # sync HW queue is kept free for the per-tile output DMAs.
            xs = cpool.tile([P, NCK, XW], BF16)
            wt = cpool.tile([P, K, NCK, COUT], BF16)
            xs_src = xs_d[:].rearrange("(ci p) t -> p ci t", p=P)
            wt_src = wt_d[:].rearrange("k p (ci d) -> p k ci d", ci=NCK)


            c2_d3 = c2_d[:].rearrange("(b t) f -> b t f", b=1)
            cbs = {}

            def cb_fetch(t):
                cb = cbpool.tile([P, CW], F32, tag="cb", name=f"cb{t}")
                nc.gpsimd.dma_start(
                    cb[:], c2_d3[0, t : t + 1, :].partition_broadcast(P)
                )
                cbs[t] = cb

            warm = cpool.tile([P, P + COUT], BF16)
            nc.scalar.dma_start(warm[:], warm_d[:])
            for i in range(4):
                wp = pwarm.tile([P, COUT], F32, tag="warm", name="wp")
                nc.tensor.matmul(
                    wp[:], warm[:, 0:P], warm[:, P : P + COUT],
                    start=True, stop=True,
                )
            nc.sync.dma_start(xs[:, :, 0:130], xs_src[:, :, 0:130])
            bias_bc = cpool.tile([P, COUT], F32)
            nc.sync.dma_start(bias_bc[:], bias_d[:].partition_broadcast(P))
            diag7 = cpool.tile([P, CW], F32)
            nc.sync.dma_start(diag7[:], diag_d[:])
            nc.gpsimd.dma_start(wt[:, 0, :, :], wt_src[:, 0, :, :])
            nc.gpsimd.dma_start(xs[:, :, 130:736], xs_src[:, :, 130:736])
            nc.gpsimd.dma_start(wt[:, 1, :, :], wt_src[:, 1, :, :])
            nc.gpsimd.dma_start(wt[:, 2, :, :], wt_src[:, 2, :, :])
            nc.gpsimd.dma_start(xs[:, :, 736:1460], xs_src[:, :, 736:1460])
            nc.gpsimd.dma_start(wt[:, 3, :, :], wt_src[:, 3, :, :])
            nc.gpsimd.dma_start(wt[:, 4, :, :], wt_src[:, 4, :, :])
            nc.gpsimd.dma_start(xs[:, :, 1460:XW], xs_src[:, :, 1460:XW])
            nc.gpsimd.dma_start(wt[:, 5, :, :], wt_src[:, 5, :, :])
            nc.gpsimd.dma_start(wt[:, 6, :, :], wt_src[:, 6, :, :])
            for t in range(TILES):
                cb_fetch(t)

            for grp in groups:
                # ---- stage 1, k-outer: Y_t[dp,k,d] for each tile band ----
                # S-builds are interleaved between k-sweeps so they sit early
                # in the strict-FIFO DVE/ACT queues and are ready by stage 2.
                ys = {}
                for t in grp:
                    ys[t] = ypool.tile([P, K, COUT], BF16, tag="y_sb", name=f"y{t}")
                ss = {}

                def build_s(t):
                    cb = cbs[t]
                    s_sb = stpool.tile([P, CW], BF16, tag="s_sb", name=f"s{t}")
                    reg = slice(0, K * OT)
                    nc.vector.tensor_tensor(
                        cb[:, reg], cb[:, reg], diag7[:, reg],
                        mybir.AluOpType.subtract,
                    )
                    nc.scalar.activation(
                        cb[:, reg], cb[:, reg],
                        mybir.ActivationFunctionType.Abs,
                    )
                    nc.scalar.activation(
                        s_sb[:, reg], cb[:, reg],
                        mybir.ActivationFunctionType.Relu,
                        bias=1.0, scale=-1.0,
                    )
                    ss[t] = s_sb

                for k in range(K):
                    for t in grp:
                        o0 = t * OT
                        yp = psy.tile([P, COUT], F32, tag="yp", name=f"yp{k}_{t}")
                        for ci in range(NCK):
                            nc.tensor.matmul(
                                yp[:],
                                xs[:, ci, o0 : o0 + P],
                                wt[:, k, ci, :],
                                start=(ci == 0), stop=(ci == NCK - 1),
                            )
                        # evict; fold bias into k=0 (sum_p S_0[p,o] == 1)
                        if k == 0:
                            nc.vector.tensor_tensor(
                                ys[t][:, 0, :], yp[:], bias_bc[:],
                                mybir.AluOpType.add,
                            )
                        elif k in (1, 2, 3):
                            nc.vector.tensor_copy(ys[t][:, k, :], yp[:])
                        else:
                            nc.scalar.copy(ys[t][:, k, :], yp[:])
                    if k < len(grp):
                        build_s(grp[k])
                for i in range(K, len(grp)):
                    build_s(grp[i])

                # ---- stage 2: out[o, d] = sum_k S_k^T Y_k ----
                for t in grp:
                    o0 = t * OT
                    op = pso.tile([P, COUT], F32, tag="op", name=f"op{t}")
                    o_sb = opool.tile([P, COUT], BF16, tag="o_sb", name=f"o{t}")
                    if t < TILES - 1:
                        for k in range(K):
                            nc.tensor.matmul(
                                op[:OT],
                                ss[t][:, k * OT : k * OT + OT],
                                ys[t][:, k, :],
                                start=(k == 0), stop=(k == K - 1),
                            )
                        if t % 2 == 0:
                            nc.scalar.copy(o_sb[:OT], op[:OT])
                        else:
                            nc.vector.tensor_copy(o_sb[:OT], op[:OT])
                        nc.sync.dma_start(out_d[o0 : o0 + OT, :], o_sb[:OT])
                    else:
                        # last tile: run stage 2 in d-halves so the eviction
                        # and output DMA of half 0 overlap half 1's matmuls,
                        # shortening the post-stream tail; outs ride the
                        # by-now-idle gpsimd queue.
                        H2 = COUT // 2
                        for h in (0, 1):
                            dsl = slice(h * H2, (h + 1) * H2)
                            for k in range(K):
                                nc.tensor.matmul(
                                    op[:OT, dsl],
                                    ss[t][:, k * OT : k * OT + OT],
                                    ys[t][:, k, dsl],
                                    start=(k == 0), stop=(k == K - 1),
                                )
                            if h == 0:
                                nc.scalar.copy(o_sb[:OT, dsl], op[:OT, dsl])
                            else:
                                nc.vector.tensor_copy(o_sb[:OT, dsl], op[:OT, dsl])
                            nc.gpsimd.dma_start(
                                out_d[o0 : o0 + OT, dsl], o_sb[:OT, dsl]
                            )

    nc.compile()
    return nc


def _install_axon_ntff_hook():
    """Provide antenv.axon_hooks (absent on this image) so that
    run_bass_kernel_spmd(trace=True) can capture NTFF profiles via the
    axon .so's C ABI.  Mirrors trn_agent_boot.trn_boot."""
    import contextlib
    import ctypes
    import types

    try:
        from antenv.axon_hooks import set_axon_ntff_profile_hook  # noqa: F401
        return
    except ImportError:
        pass

    so_path = "/opt/axon/libaxon_pjrt.so"
    if not os.path.exists(so_path):
        return
    lib = ctypes.CDLL(so_path)
    if not hasattr(lib, "axon_start_nrt_profile"):
        return
    lib.axon_start_nrt_profile.argtypes = [
        ctypes.POINTER(ctypes.c_int64), ctypes.c_size_t,
    ]
    lib.axon_start_nrt_profile.restype = ctypes.c_int64
    lib.axon_stop_nrt_profile.argtypes = [ctypes.c_char_p]
    lib.axon_stop_nrt_profile.restype = ctypes.c_int64

    @contextlib.contextmanager
    def _hook(output_dir, device_ids):
        import jax

        jax.devices()
        if device_ids:
            ids = (ctypes.c_int64 * len(device_ids))(*device_ids)
            rc = lib.axon_start_nrt_profile(ids, len(device_ids))
        else:
            rc = lib.axon_start_nrt_profile(None, 0)
        if rc != 0:
            raise RuntimeError(f"axon_start_nrt_profile rc={rc}")
        try:
            yield
        finally:
            n = lib.axon_stop_nrt_profile(str(output_dir).encode())
            print(f"ntff profile: {n} file(s) written to {output_dir}")

    box = {"h": _hook}
    mod = types.ModuleType("antenv.axon_hooks")
    mod.get_axon_ntff_profile_hook = lambda: box["h"]
    mod.set_axon_ntff_profile_hook = lambda h: box.__setitem__("h", h)
    import antenv

    sys.modules["antenv.axon_hooks"] = mod
    antenv.axon_hooks = mod

    # zero-egress env: skip the artifact upload in the trace path
    from concourse import bass_utils as _bu

    _bu.upload_artifacts = lambda d: f"local:{d}"


def _consts():
    # diag7[dp, k*OT + j] = dp - j   (j = o - o0), packed flat
    dp = np.arange(P, dtype=np.float32).reshape(P, 1)
    j = np.arange(OT, dtype=np.float32).reshape(1, OT)
    blk = dp - j  # [P, OT]
    diag7 = np.zeros((P, CW), dtype=np.float32)
    for k in range(K):
        diag7[:, k * OT : k * OT + OT] = blk
    warm = np.ones((P, P + COUT), dtype=ml_dtypes.bfloat16)
    return diag7, warm


def kernel(x, offsets, weight, bias, _trace=False, _trace_kwargs=None):
    x = np.asarray(x, dtype=np.float32)
    offsets = np.asarray(offsets, dtype=np.float32)
    weight = np.asarray(weight, dtype=np.float32)
    bias = np.asarray(bias, dtype=np.float32)

    if "nc" not in _prog_cache:
        _prog_cache["nc"] = _build_program()
    nc = _prog_cache["nc"]

    # wt slab [k, p, ci*d] = weight[d, ci*128+p, k]: 4KB-contiguous rows on
    # both DMA sides -> few, large descriptors on the gpsimd SW queue
    w_t = np.ascontiguousarray(
        np.transpose(weight, (2, 1, 0))
        .reshape(K, NCK, P, COUT)
        .transpose(0, 2, 1, 3)
        .reshape(K, P, NCK * COUT)
        .astype(ml_dtypes.bfloat16)
    )
    biasr = np.ascontiguousarray(bias.reshape(1, COUT))
    diag7, warm = _consts()
    kk = np.arange(K, dtype=np.float32).reshape(1, K)

    in_maps = []
    for core in range(8):
        b, half = core // 2, core % 2
        o_off = half * HALF
        xs = np.zeros((CIN, XW), dtype=ml_dtypes.bfloat16)
        xw = min(L - o_off, XW)
        xs[:, :xw] = x[b][:, o_off : o_off + xw].astype(ml_dtypes.bfloat16)
        # c2[t, k*OT + j] = clamp(k + off[o0+j, k], 0, 6), per-tile rows
        ow = min(OUT_LEN - o_off, OPAD)
        cpad = np.zeros((OPAD, K), dtype=np.float32)
        cpad[:ow] = np.clip(offsets[b, 0, o_off : o_off + ow, :] + kk, 0.0, 6.0)
        c2 = np.zeros((TILES, CW), dtype=np.float32)
        c2[:, : K * OT] = (
            cpad.reshape(TILES, OT, K).transpose(0, 2, 1).reshape(TILES, K * OT)
        )
        in_maps.append(
            {
                "xs": xs, "wt": w_t, "c2": c2, "diag7": diag7,
                "biasr": biasr, "warm": warm,
            }
        )

    if _trace:
        _install_axon_ntff_hook()
    try:
        res = run_bass_kernel_spmd(
            nc, in_maps, core_ids=list(range(8)),
            trace=_trace, **(_trace_kwargs or {}),
        )
    except Exception:
        # transient runtime faults have been observed; one retry
        res = run_bass_kernel_spmd(
            nc, in_maps, core_ids=list(range(8)),
            trace=_trace, **(_trace_kwargs or {}),
        )

    out = np.empty((B, COUT, OUT_LEN), dtype=np.float32)
    for core in range(8):
        b, half = core // 2, core % 2
        o_off = half * HALF
        out[b, :, o_off : o_off + HALF] = (
            res.results[core]["out"][:HALF, :].astype(np.float32).T
        )
    if _trace:
        _prog_cache["last_exec_time_ns"] = res.exec_time_ns
    return out


# revision 20
# speedup vs baseline: 1.0089x; 1.0089x over previous
"""DeformConv1d Trainium2 Bass kernel (optimized).

Problem: x[4,512,4096] f32, offsets[4,1,4090,7] f32, weight[512,512,7], bias[512]
  T[b,o,k]   = clamp(o + k + offsets[b,0,o,k], o, o+6)
  samp[b,c,o,k] = linear-interp of x[b,c,:] at T
  out[b,d,o] = sum_{c,k} samp[b,c,o,k] * weight[d,c,k] + bias[d]

Key identity: the clamp keeps every sample inside [o, o+6], so output o only
touches taps p in [o, o+7], and the tap weight of p is relu(1 - |p - T|).
With an o-tile of 121 the tap band is exactly 128 wide:

  out[o,d] = sum_{k, dp<128} S_k[dp, o] * Y[o0+dp, k, d] (+ bias via Y_0)
    S_k[dp,o] = relu(1 - |(dp - (o-o0)) - c_k[o]|),  c_k[o] = clamp(k + off, 0, 6)
    Y[p,k,d]  = sum_c x[c, p] * weight[d, c, k]

Both stages are dense bf16 matmuls (f32 PSUM accumulate); the 595-matmul
stream runs at the 216ns/matmul N=512 streaming roofline.  Design notes:
  - c_k is precomputed on host into per-tile rows (c2); a 0-stride DMA
    expands each row across partitions; S = relu(1-|c-diag|) on DVE+ACT.
  - Stage 1 runs k-outer over groups of 6 o-tiles so the first tile does
    not need the whole 3.7MB weight within one tile period; S-builds are
    interleaved between k-sweeps so they sit early in the strict-FIFO
    DVE/ACT queues and are ready when stage 2 arrives.
  - All bulk traffic rides the gpsimd SW DGE queue, which sustains
    ~400GB/s with per-partition-contiguous "slab" layouts (the HW DGE
    queues measure only ~50GB/s here).  Weight is host-packed as per-k
    slabs [k, p, ci*d] (4KB contiguous rows on both DMA sides).
  - A 164KB warm tile on the scalar HW queue feeds 4 dummy matmuls that
    hold the PE busy through the HAM activity window, so the real stream
    runs at 2.4GHz (warm) from the start.
  - bias is folded into the k=0 Y eviction (sum_p S_k[p,o] == 1); output
    is stored bf16 (host upcasts); the last tile's stage 2 runs in
    d-halves so its eviction/DMA overlaps the final matmuls.
Sharding: 8 cores = 4 batches x 2 halves of out_len (data parallel, no comm).
"""

import os
import sys

import ml_dtypes
import numpy as np

for _p in ("/opt/trn_rl_repo", os.path.expanduser("~/.axon_site/_ro/trn_rl_repo")):
    if os.path.isdir(_p) and _p not in sys.path:
        sys.path.insert(0, _p)

import concourse.mybir as mybir
import concourse.tile as tile
from concourse import bacc
from concourse.bass_utils import run_bass_kernel_spmd

B, CIN, COUT, L, K = 4, 512, 512, 4096, 7
OUT_LEN = 4090
HALF = 2045          # out positions per core (2 halves per batch)
OT = 121             # o-tile size -> tap band = OT + 7 = 128
TILES = 17           # 17 * 121 = 2057 >= 2045
OPAD = TILES * OT    # 2057 padded out positions per core
XW = (TILES - 1) * OT + 128  # 2064: rightmost x column any tile reads
P = 128
NCK = CIN // P       # 4 c-chunks
CW = K * OT + 1      # 848: packed c/diag/S row width (847 used, +1 pad)
G = 8                # o-tiles per stage-1 k-outer group
F32 = mybir.dt.float32
BF16 = mybir.dt.bfloat16

_prog_cache = {}


def _build_program():
    nc = bacc.Bacc("TRN2", target_bir_lowering=False, debug=False)

    xs_d = nc.dram_tensor("xs", [CIN, XW], BF16, kind="ExternalInput")
    wt_d = nc.dram_tensor("wt", [K, P, NCK * COUT], BF16, kind="ExternalInput")
    c2_d = nc.dram_tensor("c2", [TILES, CW], F32, kind="ExternalInput")
    diag_d = nc.dram_tensor("diag7", [P, CW], F32, kind="ExternalInput")
    bias_d = nc.dram_tensor("biasr", [1, COUT], F32, kind="ExternalInput")
    warm_d = nc.dram_tensor("warm", [P, P + COUT], BF16, kind="ExternalInput")
    out_d = nc.dram_tensor("out", [OPAD, COUT], BF16, kind="ExternalOutput")

    groups = [list(range(g, min(g + G, TILES))) for g in range(0, TILES, G)]

    with tile.TileContext(nc) as tc:
        with (
            tc.tile_pool(name="const", bufs=1) as cpool,
            tc.tile_pool(name="cbt", bufs=12) as cbpool,
            tc.tile_pool(name="stiles", bufs=G + 1) as stpool,
            tc.tile_pool(name="ytiles", bufs=G + 1) as ypool,
            tc.tile_pool(name="otiles", bufs=3) as opool,
            tc.tile_pool(name="pwarm", bufs=1, space="PSUM") as pwarm,
            tc.tile_pool(name="psy", bufs=4, space="PSUM") as psy,
            tc.tile_pool(name="pso", bufs=3, space="PSUM") as pso,
        ):
            # ---- bulk inputs + broadcasts all on the gpsimd SW queue
            # (measured ~250GB/s vs ~50GB/s on the HW DGE queues), issued in
            # group-0 consumption-deadline order; consts on scalar HW queue;
            # sync HW queue is kept free for the per-tile output DMAs.
            xs = cpool.tile([P, NCK, XW], BF16)
            wt = cpool.tile([P, K, NCK, COUT], BF16)
            xs_src = xs_d[:].rearrange("(ci p) t -> p ci t", p=P)
            wt_src = wt_d[:].rearrange("k p (ci d) -> p k ci d", ci=NCK)


            c2_d3 = c2_d[:].rearrange("(b t) f -> b t f", b=1)
            cbs = {}

            def cb_fetch(t):
                cb = cbpool.tile([P, CW], F32, tag="cb", name=f"cb{t}")
                nc.gpsimd.dma_start(
                    cb[:], c2_d3[0, t : t + 1, :].partition_broadcast(P)
                )
                cbs[t] = cb

            warm = cpool.tile([P, P + COUT], BF16)
            nc.scalar.dma_start(warm[:], warm_d[:])
            for i in range(2):
                wp = pwarm.tile([P, COUT], F32, tag="warm", name="wp")
                nc.tensor.matmul(
                    wp[:], warm[:, 0:P], warm[:, P : P + COUT],
                    start=True, stop=True,
                )
            nc.sync.dma_start(xs[:, :, 0:130], xs_src[:, :, 0:130])
            bias_bc = cpool.tile([P, COUT], F32)
            nc.sync.dma_start(bias_bc[:], bias_d[:].partition_broadcast(P))
            diag7 = cpool.tile([P, CW], F32)
            nc.sync.dma_start(diag7[:], diag_d[:])
            nc.gpsimd.dma_start(wt[:, 0, :, :], wt_src[:, 0, :, :])
            nc.gpsimd.dma_start(xs[:, :, 130:736], xs_src[:, :, 130:736])
            nc.gpsimd.dma_start(wt[:, 1, :, :], wt_src[:, 1, :, :])
            nc.gpsimd.dma_start(wt[:, 2, :, :], wt_src[:, 2, :, :])
            nc.gpsimd.dma_start(xs[:, :, 736:1460], xs_src[:, :, 736:1460])
            nc.gpsimd.dma_start(wt[:, 3, :, :], wt_src[:, 3, :, :])
            nc.gpsimd.dma_start(wt[:, 4, :, :], wt_src[:, 4, :, :])
            nc.gpsimd.dma_start(xs[:, :, 1460:XW], xs_src[:, :, 1460:XW])
            nc.gpsimd.dma_start(wt[:, 5, :, :], wt_src[:, 5, :, :])
            nc.gpsimd.dma_start(wt[:, 6, :, :], wt_src[:, 6, :, :])
            for t in range(TILES):
                cb_fetch(t)

            for grp in groups:
                # ---- stage 1, k-outer: Y_t[dp,k,d] for each tile band ----
                # S-builds are interleaved between k-sweeps so they sit early
                # in the strict-FIFO DVE/ACT queues and are ready by stage 2.
                ys = {}
                for t in grp:
                    ys[t] = ypool.tile([P, K, COUT], BF16, tag="y_sb", name=f"y{t}")
                ss = {}

                def build_s(t):
                    cb = cbs[t]
                    s_sb = stpool.tile([P, CW], BF16, tag="s_sb", name=f"s{t}")
                    reg = slice(0, K * OT)
                    nc.vector.tensor_tensor(
                        cb[:, reg], cb[:, reg], diag7[:, reg],
                        mybir.AluOpType.subtract,
                    )
                    nc.scalar.activation(
                        cb[:, reg], cb[:, reg],
                        mybir.ActivationFunctionType.Abs,
                    )
                    nc.scalar.activation(
                        s_sb[:, reg], cb[:, reg],
                        mybir.ActivationFunctionType.Relu,
                        bias=1.0, scale=-1.0,
                    )
                    ss[t] = s_sb

                for k in range(K):
                    for t in grp:
                        o0 = t * OT
                        yp = psy.tile([P, COUT], F32, tag="yp", name=f"yp{k}_{t}")
                        for ci in range(NCK):
                            nc.tensor.matmul(
                                yp[:],
                                xs[:, ci, o0 : o0 + P],
                                wt[:, k, ci, :],
                                start=(ci == 0), stop=(ci == NCK - 1),
                            )
                        # evict; fold bias into k=0 (sum_p S_0[p,o] == 1)
                        if k == 0:
                            nc.vector.tensor_tensor(
                                ys[t][:, 0, :], yp[:], bias_bc[:],
                                mybir.AluOpType.add,
                            )
                        elif k in (1, 2, 3):
                            nc.vector.tensor_copy(ys[t][:, k, :], yp[:])
                        else:
                            nc.scalar.copy(ys[t][:, k, :], yp[:])
                    if k < len(grp):
                        build_s(grp[k])
                for i in range(K, len(grp)):
                    build_s(grp[i])

                # ---- stage 2: out[o, d] = sum_k S_k^T Y_k ----
                for t in grp:
                    o0 = t * OT
                    op = pso.tile([P, COUT], F32, tag="op", name=f"op{t}")
                    o_sb = opool.tile([P, COUT], BF16, tag="o_sb", name=f"o{t}")
                    if t < TILES - 1:
                        for k in range(K):
                            nc.tensor.matmul(
                                op[:OT],
                                ss[t][:, k * OT : k * OT + OT],
                                ys[t][:, k, :],
                                start=(k == 0), stop=(k == K - 1),
                            )
                        if t % 2 == 0:
                            nc.scalar.copy(o_sb[:OT], op[:OT])
                        else:
                            nc.vector.tensor_copy(o_sb[:OT], op[:OT])
                        nc.sync.dma_start(out_d[o0 : o0 + OT, :], o_sb[:OT])
                    else:
                        # last tile: run stage 2 in d-halves so the eviction
                        # and output DMA of half 0 overlap half 1's matmuls,
                        # shortening the post-stream tail; outs ride the
                        # by-now-idle gpsimd queue.
                        H2 = COUT // 2
                        for h in (0, 1):
                            dsl = slice(h * H2, (h + 1) * H2)
                            for k in range(K):
                                nc.tensor.matmul(
                                    op[:OT, dsl],
                                    ss[t][:, k * OT : k * OT + OT],
                                    ys[t][:, k, dsl],
                                    start=(k == 0), stop=(k == K - 1),
                                )
                            if h == 0:
                                nc.scalar.copy(o_sb[:OT, dsl], op[:OT, dsl])
                            else:
                                nc.vector.tensor_copy(o_sb[:OT, dsl], op[:OT, dsl])
                            nc.gpsimd.dma_start(
                                out_d[o0 : o0 + OT, dsl], o_sb[:OT, dsl]
                            )

    nc.compile()
    return nc


def _install_axon_ntff_hook():
    """Provide antenv.axon_hooks (absent on this image) so that
    run_bass_kernel_spmd(trace=True) can capture NTFF profiles via the
    axon .so's C ABI.  Mirrors trn_agent_boot.trn_boot."""
    import contextlib
    import ctypes
    import types

    try:
        from antenv.axon_hooks import set_axon_ntff_profile_hook  # noqa: F401
        return
    except ImportError:
        pass

    so_path = "/opt/axon/libaxon_pjrt.so"
    if not os.path.exists(so_path):
        return
    lib = ctypes.CDLL(so_path)
    if not hasattr(lib, "axon_start_nrt_profile"):
        return
    lib.axon_start_nrt_profile.argtypes = [
        ctypes.POINTER(ctypes.c_int64), ctypes.c_size_t,
    ]
    lib.axon_start_nrt_profile.restype = ctypes.c_int64
    lib.axon_stop_nrt_profile.argtypes = [ctypes.c_char_p]
    lib.axon_stop_nrt_profile.restype = ctypes.c_int64

    @contextlib.contextmanager
    def _hook(output_dir, device_ids):
        import jax

        jax.devices()
        if device_ids:
            ids = (ctypes.c_int64 * len(device_ids))(*device_ids)
            rc = lib.axon_start_nrt_profile(ids, len(device_ids))
        else:
            rc = lib.axon_start_nrt_profile(None, 0)
        if rc != 0:
            raise RuntimeError(f"axon_start_nrt_profile rc={rc}")
        try:
            yield
        finally:
            n = lib.axon_stop_nrt_profile(str(output_dir).encode())
            print(f"ntff profile: {n} file(s) written to {output_dir}")

    box = {"h": _hook}
    mod = types.ModuleType("antenv.axon_hooks")
    mod.get_axon_ntff_profile_hook = lambda: box["h"]
    mod.set_axon_ntff_profile_hook = lambda h: box.__setitem__("h", h)
    import antenv

    sys.modules["antenv.axon_hooks"] = mod
    antenv.axon_hooks = mod

    # zero-egress env: skip the artifact upload in the trace path
    from concourse import bass_utils as _bu

    _bu.upload_artifacts = lambda d: f"local:{d}"


def _consts():
    # diag7[dp, k*OT + j] = dp - j   (j = o - o0), packed flat
    dp = np.arange(P, dtype=np.float32).reshape(P, 1)
    j = np.arange(OT, dtype=np.float32).reshape(1, OT)
    blk = dp - j  # [P, OT]
    diag7 = np.zeros((P, CW), dtype=np.float32)
    for k in range(K):
        diag7[:, k * OT : k * OT + OT] = blk
    warm = np.ones((P, P + COUT), dtype=ml_dtypes.bfloat16)
    return diag7, warm


def kernel(x, offsets, weight, bias, _trace=False, _trace_kwargs=None):
    x = np.asarray(x, dtype=np.float32)
    offsets = np.asarray(offsets, dtype=np.float32)
    weight = np.asarray(weight, dtype=np.float32)
    bias = np.asarray(bias, dtype=np.float32)

    if "nc" not in _prog_cache:
        _prog_cache["nc"] = _build_program()
    nc = _prog_cache["nc"]

    # wt slab [k, p, ci*d] = weight[d, ci*128+p, k]: 4KB-contiguous rows on
    # both DMA sides -> few, large descriptors on the gpsimd SW queue
    w_t = np.ascontiguousarray(
        np.transpose(weight, (2, 1, 0))
        .reshape(K, NCK, P, COUT)
        .transpose(0, 2, 1, 3)
        .reshape(K, P, NCK * COUT)
        .astype(ml_dtypes.bfloat16)
    )
    biasr = np.ascontiguousarray(bias.reshape(1, COUT))
    diag7, warm = _consts()
    kk = np.arange(K, dtype=np.float32).reshape(1, K)

    in_maps = []
    for core in range(8):
        b, half = core // 2, core % 2
        o_off = half * HALF
        xs = np.zeros((CIN, XW), dtype=ml_dtypes.bfloat16)
        xw = min(L - o_off, XW)
        xs[:, :xw] = x[b][:, o_off : o_off + xw].astype(ml_dtypes.bfloat16)
        # c2[t, k*OT + j] = clamp(k + off[o0+j, k], 0, 6), per-tile rows
        ow = min(OUT_LEN - o_off, OPAD)
        cpad = np.zeros((OPAD, K), dtype=np.float32)
        cpad[:ow] = np.clip(offsets[b, 0, o_off : o_off + ow, :] + kk, 0.0, 6.0)
        c2 = np.zeros((TILES, CW), dtype=np.float32)
        c2[:, : K * OT] = (
            cpad.reshape(TILES, OT, K).transpose(0, 2, 1).reshape(TILES, K * OT)
        )
        in_maps.append(
            {
                "xs": xs, "wt": w_t, "c2": c2, "diag7": diag7,
                "biasr": biasr, "warm": warm,
            }
        )

    if _trace:
        _install_axon_ntff_hook()
    try:
        res = run_bass_kernel_spmd(
            nc, in_maps, core_ids=list(range(8)),
            trace=_trace, **(_trace_kwargs or {}),
        )
    except Exception:
        # transient runtime faults have been observed; one retry
        res = run_bass_kernel_spmd(
            nc, in_maps, core_ids=list(range(8)),
            trace=_trace, **(_trace_kwargs or {}),
        )

    out = np.empty((B, COUT, OUT_LEN), dtype=np.float32)
    for core in range(8):
        b, half = core // 2, core % 2
        o_off = half * HALF
        out[b, :, o_off : o_off + HALF] = (
            res.results[core]["out"][:HALF, :].astype(np.float32).T
        )
    if _trace:
        _prog_cache["last_exec_time_ns"] = res.exec_time_ns
    return out
